# revision 14
# baseline (speedup 1.0000x reference)
"""GQA attention kernel for Trainium2, 8 NeuronCores.

Sharding: core c handles batch b=c//2 and group-half gh=c%2 (4 KV groups =
16 Q heads, full sequence). Each core computes its Q/K/V projections,
attention (scores in transposed orientation so softmax-normalize and the
attn@v contraction share one layout), and a row-sharded partial of the fc
output. Host sums the two fc partials per batch and transposes attention
maps back to [Sq, Sk].

Matmuls run as float32r (fp22 mantissa, full PE rate); inputs are
pre-rounded to fp22 on host with round-to-nearest-even so the on-chip
truncation is lossless.
"""
import sys
sys.path.insert(0, "/opt/trn_rl_repo")
import numpy as np
import concourse.bass as bass
import concourse.bacc as bacc
import concourse.tile as tile
from concourse import mybir
from concourse.bass_utils import run_bass_kernel_spmd

f32 = mybir.dt.float32
f32r = mybir.dt.float32r
AF = mybir.ActivationFunctionType

B, S, D = 4, 1024, 2048
H, G, HD = 32, 8, 64
NCORE = 8
GH_Q = 1024          # q cols per core (16 heads)
GH_KV = 256          # kv cols per core (4 groups)
NH = 16              # heads per core
NG = 4               # groups per core
KC = D // 128        # 16 contraction chunks for projections
SC = S // 128        # 8 sk chunks
SCALE = 0.125        # 1/sqrt(64)

# Head->qT slot permutation: a K=64 matmul needs lhsT (kT, parity of the
# group) and rhs (qT) at the same base partition, so heads of even groups
# go to row-offset 0 and odd groups to row-offset 64 within their chunk.
EVEN_HEADS = [0, 1, 2, 3, 8, 9, 10, 11]     # groups 0, 2
ODD_HEADS = [4, 5, 6, 7, 12, 13, 14, 15]    # groups 1, 3
Q_SLOT = {}
for _m in range(8):
    Q_SLOT[EVEN_HEADS[_m]] = (_m, 0)
    Q_SLOT[ODD_HEADS[_m]] = (_m, 64)
Q_COL_PERM = np.concatenate(
    [np.arange(64 * h, 64 * h + 64)
     for _m in range(8) for h in (EVEN_HEADS[_m], ODD_HEADS[_m])])

_CACHED = None


def _round_fp22(a):
    """Round fp32 mantissa to 13 bits (fp32r's fp22), round-to-nearest-even."""
    b = np.ascontiguousarray(a, dtype=np.float32).view(np.uint32)
    drop = 10
    half = np.uint32(1 << (drop - 1))
    lsb = (b >> np.uint32(drop)) & np.uint32(1)
    r = b + (half - np.uint32(1)) + lsb
    r &= np.uint32(~np.uint32((1 << drop) - 1))
    return r.view(np.float32)


def _build():
    nc = bacc.Bacc("TRN2", target_bir_lowering=False, debug=False,
                   num_devices=NCORE)
    xT_d = nc.dram_tensor("xT", [D, S], f32r, kind="ExternalInput")
    wq_d = nc.dram_tensor("wq", [D, GH_Q], f32r, kind="ExternalInput")
    wk_d = nc.dram_tensor("wk", [D, GH_KV], f32r, kind="ExternalInput")
    wv_d = nc.dram_tensor("wv", [D, GH_KV], f32r, kind="ExternalInput")
    fcw_d = nc.dram_tensor("fcw", [GH_Q, D], f32r, kind="ExternalInput")
    qb_d = nc.dram_tensor("qb", [GH_Q], f32, kind="ExternalInput")
    kb_d = nc.dram_tensor("kb", [GH_KV], f32, kind="ExternalInput")
    attn_d = nc.dram_tensor("attn", [NH, S, S], f32r, kind="ExternalOutput")
    fc_d = nc.dram_tensor("fc", [S, D], f32, kind="ExternalOutput")

    with tile.TileContext(nc) as tc:
        with (
            tc.tile_pool(name="const", bufs=1) as cpool,
            tc.tile_pool(name="qT", bufs=1) as qpool,
            tc.tile_pool(name="kT", bufs=1) as kpool,
            tc.tile_pool(name="vaug", bufs=1) as vpool,
            tc.tile_pool(name="ctx", bufs=1) as ctxpool,
        ):
            qb_t = cpool.tile([128, 8], f32)
            kb_t = cpool.tile([128, 2], f32)
            nc.sync.dma_start(qb_t[:], qb_d.ap().rearrange("(t p) -> p t", p=128))
            nc.sync.dma_start(kb_t[:], kb_d.ap().rearrange("(t p) -> p t", p=128))

            qT = qpool.tile([128, 8, S], f32r)     # row 128t+p = q col
            kT = kpool.tile([128, 2, S], f32r)
            vaug = vpool.tile([128, SC, NG * 65], f32r)  # [sk%128, skc, g*65+j]

            # ---------------- Phase P: projections ----------------
            with (
                tc.tile_pool(name="xt", bufs=1) as xpool,
                tc.tile_pool(name="w", bufs=1) as wpool,
                tc.tile_pool(name="psp", bufs=4, space=bass.MemorySpace.PSUM) as psp,
            ):
                xt = xpool.tile([128, KC, S], f32r)
                nc.sync.dma_start(xt[:], xT_d.ap().rearrange("(k p) s -> p k s", p=128))

                # ones columns of vaug (denominator accumulator rows)
                for sv in range(SC):
                    for g in range(NG):
                        nc.scalar.activation(
                            vaug[:, sv, 65 * g + 64:65 * g + 65],
                            kb_t[:, 0:1], AF.Identity, scale=0.0, bias=1.0)

                # K proj first (smallest weights -> earliest PE start)
                wk_t = wpool.tile([128, KC, GH_KV], f32r, tag="w")
                nc.sync.dma_start(wk_t[:], wk_d.ap().rearrange("(k p) c -> p k c", p=128))
                for c in range(2):
                    for sq in range(2):
                        ps = psp.tile([128, 512], f32, tag="psp")
                        for k in range(KC):
                            nc.tensor.matmul(
                                ps[:], wk_t[:, k, 128 * c:128 * (c + 1)],
                                xt[:, k, 512 * sq:512 * (sq + 1)],
                                start=(k == 0), stop=(k == KC - 1))
                        nc.scalar.activation(kT[:, c, 512 * sq:512 * (sq + 1)], ps[:],
                                             AF.Identity, bias=kb_t[:, c:c + 1])

                # V proj: natural orientation [sk, col]
                wv_t = wpool.tile([128, KC, GH_KV], f32r, tag="w")
                nc.sync.dma_start(wv_t[:], wv_d.ap().rearrange("(k p) c -> p k c", p=128))
                for sv in range(SC):
                    ps = psp.tile([128, GH_KV], f32, tag="psp")
                    for k in range(KC):
                        nc.tensor.matmul(
                            ps[:], xt[:, k, 128 * sv:128 * (sv + 1)],
                            wv_t[:, k, :],
                            start=(k == 0), stop=(k == KC - 1))
                    for g in range(NG):
                        nc.scalar.activation(
                            vaug[:, sv, 65 * g:65 * g + 64],
                            ps[:, 64 * g:64 * (g + 1)], AF.Identity)

                # Q proj, in two column-halves to bound SBUF residency
                wq_r = wq_d.ap().rearrange("(k p) c -> p k c", p=128)
                for half in range(2):
                    wq_t = wpool.tile([128, KC, GH_Q // 2], f32r, tag="w")
                    nc.sync.dma_start(wq_t[:], wq_r[:, :, 512 * half:512 * (half + 1)])
                    for mh in range(4):
                        m = 4 * half + mh
                        for sq in range(2):
                            ps = psp.tile([128, 512], f32, tag="psp")
                            for k in range(KC):
                                nc.tensor.matmul(
                                    ps[:], wq_t[:, k, 128 * mh:128 * (mh + 1)],
                                    xt[:, k, 512 * sq:512 * (sq + 1)],
                                    start=(k == 0), stop=(k == KC - 1))
                            nc.scalar.activation(qT[:, m, 512 * sq:512 * (sq + 1)],
                                                 ps[:], AF.Identity,
                                                 bias=qb_t[:, m:m + 1])

            # ---------------- Phase A: attention ----------------
            with (
                tc.tile_pool(name="exp", bufs=2) as epool,
                tc.tile_pool(name="bc", bufs=2) as bcpool,
                tc.tile_pool(name="pss", bufs=4, space=bass.MemorySpace.PSUM) as pss,
                tc.tile_pool(name="psc", bufs=2, space=bass.MemorySpace.PSUM) as psc,
            ):
                ctxT = ctxpool.tile([128, 8, S], f32r)

                for h in range(NH):
                    g = h // 4            # local group
                    qch, qrow = Q_SLOT[h]             # permuted q layout
                    cch, crow = h // 2, 64 * (h % 2)  # ctxT in head order
                    expT = epool.tile([128, SC, S], f32r, tag="exp")
                    for c in range(SC):
                        for sq in range(2):
                            ps = pss.tile([128, 512], f32, tag="pss")
                            nc.tensor.matmul(
                                ps[:],
                                kT[64 * (g % 2):64 * (g % 2) + 64, g // 2,
                                   128 * c:128 * (c + 1)],
                                qT[qrow:qrow + 64, qch, 512 * sq:512 * (sq + 1)],
                                start=True, stop=True)
                            nc.scalar.activation(
                                expT[:, c, 512 * sq:512 * (sq + 1)], ps[:],
                                AF.Exp, scale=SCALE)

                    bc_t = bcpool.tile([128, S], f32, tag="bc")
                    r_t = bcpool.tile([1, S], f32, tag="r")
                    cps_list = []
                    for sq in range(2):
                        cps = psc.tile([128, 512], f32, tag="psc")
                        cps_list.append(cps)
                        for c in range(SC):
                            nc.tensor.matmul(
                                cps[0:65, :], vaug[:, c, 65 * g:65 * (g + 1)],
                                expT[:, c, 512 * sq:512 * (sq + 1)],
                                start=(c == 0), stop=(c == SC - 1))
                        nc.vector.reciprocal(r_t[0:1, 512 * sq:512 * (sq + 1)],
                                             cps[64:65, :])
                    nc.gpsimd.partition_broadcast(bc_t[:], r_t[0:1, :])
                    # normalized context -> ctxT rows 64h..64h+64
                    for sq in range(2):
                        nc.vector.tensor_mul(
                            ctxT[crow:crow + 64, cch, 512 * sq:512 * (sq + 1)],
                            cps_list[sq][0:64, :], bc_t[0:64, 512 * sq:512 * (sq + 1)])
                    # normalize attn in place (DVE 5 chunks, GpSimd 3)
                    for c in range(SC):
                        eng = nc.vector if c < 5 else nc.gpsimd
                        eng.tensor_mul(expT[:, c, :], expT[:, c, :], bc_t[:, :])
                    nc.sync.dma_start(
                        attn_d[h].rearrange("(c p) q -> p c q", p=128), expT[:])

            # ---------------- Phase F: fc partial ----------------
            with (
                tc.tile_pool(name="fcw", bufs=1) as fwpool,
                tc.tile_pool(name="fo", bufs=2) as fopool,
                tc.tile_pool(name="psf", bufs=4, space=bass.MemorySpace.PSUM) as psf,
            ):
                fcw_t = fwpool.tile([128, 8, D], f32r)
                nc.sync.dma_start(fcw_t[:],
                                  fcw_d.ap().rearrange("(k p) c -> p k c", p=128))
                for m in range(8):
                    fo = fopool.tile([128, D], f32, tag="fo")
                    for n in range(4):
                        ps = psf.tile([128, 512], f32, tag="psf")
                        for k in range(8):
                            nc.tensor.matmul(
                                ps[:], ctxT[:, k, 128 * m:128 * (m + 1)],
                                fcw_t[:, k, 512 * n:512 * (n + 1)],
                                start=(k == 0), stop=(k == 7))
                        nc.scalar.activation(fo[:, 512 * n:512 * (n + 1)], ps[:],
                                             AF.Identity)
                    nc.sync.dma_start(fc_d[128 * m:128 * (m + 1), :], fo[:])

    nc.compile()
    return nc


def _make_in_maps(x, wq_w, wq_b, wk_w, wk_b, wv_w, fc_w):
    in_maps = []
    for c in range(NCORE):
        b, gh = c // 2, c % 2
        in_maps.append({
            "xT": _round_fp22(x[b].T),
            "wq": _round_fp22(wq_w[:, GH_Q * gh:GH_Q * (gh + 1)][:, Q_COL_PERM]),
            "wk": _round_fp22(wk_w[:, GH_KV * gh:GH_KV * (gh + 1)]),
            "wv": _round_fp22(wv_w[:, GH_KV * gh:GH_KV * (gh + 1)]),
            "fcw": _round_fp22(fc_w[GH_Q * gh:GH_Q * (gh + 1), :]),
            "qb": np.ascontiguousarray(wq_b[GH_Q * gh:GH_Q * (gh + 1)][Q_COL_PERM]),
            "kb": np.ascontiguousarray(wk_b[GH_KV * gh:GH_KV * (gh + 1)]),
        })
    return in_maps


def kernel(**inputs):
    global _CACHED
    x = np.asarray(inputs["x"], np.float32)
    wq_w = np.asarray(inputs["wq_w"], np.float32)
    wq_b = np.asarray(inputs["wq_b"], np.float32)
    wk_w = np.asarray(inputs["wk_w"], np.float32)
    wk_b = np.asarray(inputs["wk_b"], np.float32)
    wv_w = np.asarray(inputs["wv_w"], np.float32)
    wv_b = np.asarray(inputs["wv_b"], np.float32)
    fc_w = np.asarray(inputs["fc_w"], np.float32)
    fc_b = np.asarray(inputs["fc_b"], np.float32)

    if _CACHED is None:
        _CACHED = _build()
    nc = _CACHED

    in_maps = _make_in_maps(x, wq_w, wq_b, wk_w, wk_b, wv_w, fc_w)

    res = run_bass_kernel_spmd(nc, in_maps, list(range(NCORE)))

    out = np.empty((B, S, D), np.float32)
    attn = np.empty((B, H, S, S), np.float32)
    # v-bias correction: ctx gets +bv (attn rows sum to 1) -> + bv_exp @ fc_w
    bv_exp = np.repeat(wv_b.reshape(G, HD), H // G, axis=0).reshape(D)
    bias_all = bv_exp @ fc_w + fc_b
    for b in range(B):
        out[b] = res.results[2 * b]["fc"] + res.results[2 * b + 1]["fc"] + bias_all
    for c in range(NCORE):
        b, gh = c // 2, c % 2
        a = res.results[c]["attn"]          # [16, Sk, Sq]
        attn[b, NH * gh:NH * (gh + 1)] = np.swapaxes(a, 1, 2)
    return out, attn
